# revision 1
# baseline (speedup 1.0000x reference)
"""Mamba-core (4-layer) Trainium2 Bass kernel.

Sharding: data-parallel over batch B=8 across 8 NeuronCores (one sample per
core, zero collectives).  Per core, all activations live in SBUF in
[feature, time] layout:

  - in_proj + causal depthwise conv are fused: conv taps are folded into 4
    time-shifted accumulating matmuls (PSUM accumulation over taps).
  - dt = softplus(...) and SiLU gates run on the scalar (ACT) engine with
    per-partition bias vectors.
  - The selective scan runs as native `tensor_tensor_scan` instructions
    (h = a*h + b along the time axis, fp32 internal state), one [128, 512]
    tile per (d_inner-half, state-n, time-chunk).
  - a = exp(-(n+1)*dt) comes straight from the ACT engine (Exp with
    scale=-(n+1)).
  - b = dtu * B_n and the readout h * C_n need B/C rows broadcast across
    partitions: rows are replicated with K=1 ones-matmuls on the tensor
    engine (PSUM holds the replicated rows).
  - y = sum_n C_n*h_n is accumulated in PSUM via identity matmuls.
"""

import os
import numpy as np

DM = 128        # d_model
DI = 256        # d_inner
NDH = 2         # d_inner halves of 128
NST = 16        # d_state
RNK = 8         # dt_rank
L = 4096
LAYERS = 4
DCONV = 4
CH = 512        # time chunk (one PSUM bank)
NCH = L // CH   # 8
QCH = 2         # chunks per quarter (y-acc PSUM granularity)
NQ = NCH // QCH  # 4 quarters
B = 8
NCORES = 8

F32 = "float32"
BF16 = "bfloat16"

# dtype config (flip these for perf/precision trades)
DT_DT = BF16    # dt tensor
DTU_DT = BF16   # dtu tensor
SZ_DT = F32     # silu(z) tensor
A_DT = F32      # scan decay operand
BT_DT = F32     # scan input operand
H_DT = F32      # scan output
TMP_DT = F32    # readout product


def prep_weights(inputs):
    """Host-side weight preprocessing (numpy, tiny)."""
    in_w = inputs["in_proj_w"]    # [4, 512, 128]
    cw = inputs["conv_w"]         # [4, 256, 4]
    cb = inputs["conv_b"]         # [4, 256]
    xp_w = inputs["x_proj_w"]     # [4, 40, 256]
    dtp_w = inputs["dt_proj_w"]   # [4, 256, 8]
    dtp_b = inputs["dt_proj_b"]   # [4, 256]
    Dp = inputs["D"]              # [4, 256]
    out_w = inputs["out_proj_w"]  # [4, 128, 256]

    wz = np.ascontiguousarray(np.transpose(in_w[:, DI:, :], (0, 2, 1)))  # [4,128,256]
    # conv folded into in_proj: wxa[l, kd, k*DI+m] = cw[l, m, k] * in_w[l, m, kd]
    wxa = np.einsum("lmk,lmd->ldkm", cw, in_w[:, :DI, :])                # [4,128,4,256]
    wxa = np.ascontiguousarray(wxa.reshape(LAYERS, DM, DCONV * DI))
    # wxp[l, ksub, dh*96 + seg]: x_proj output padded to M=96 so the PSUM
    # splits land on 32-aligned partitions: dtraw @ 0:8, Bm @ 32:48, Cm @ 64:80
    wxp_t = np.transpose(xp_w.reshape(LAYERS, 40, NDH, DM), (0, 3, 2, 1))  # [l,ksub,dh,40]
    wxp = np.zeros((LAYERS, DM, NDH, 96), np.float32)
    wxp[:, :, :, 0:RNK] = wxp_t[:, :, :, 0:RNK]
    wxp[:, :, :, 32:32 + NST] = wxp_t[:, :, :, RNK:RNK + NST]
    wxp[:, :, :, 64:64 + NST] = wxp_t[:, :, :, RNK + NST:RNK + 2 * NST]
    wxp = np.ascontiguousarray(wxp.reshape(LAYERS, DM, NDH * 96))
    wdt = np.ascontiguousarray(np.transpose(dtp_w, (0, 2, 1)))           # [4,8,256]
    # wo[l, ksub, dh*128+m] = out_w[l, m, dh*128+ksub]
    wo = np.transpose(out_w.reshape(LAYERS, DM, NDH, DM), (0, 3, 2, 1))
    wo = np.ascontiguousarray(wo.reshape(LAYERS, DM, NDH * DM))
    vecs = np.zeros((LAYERS, DM, 6), np.float32)
    for dh in range(NDH):
        s = slice(dh * DM, (dh + 1) * DM)
        vecs[:, :, 0 + dh] = cb[:, s]
        vecs[:, :, 2 + dh] = dtp_b[:, s]
        vecs[:, :, 4 + dh] = Dp[:, s]
    import ml_dtypes
    # selp[32+k or 64+k, n*128+p] = 1 iff k == n — row-n replicator lhsT,
    # placed at partition bases 32 and 64 so lhsT base matches the rhs base
    # (Bm rows live at pjs[32:48], Cm rows at pjs[64:80]).
    sel = np.zeros((80, NST * DM), np.float32)
    for n in range(NST):
        sel[32 + n, n * DM:(n + 1) * DM] = 1.0
        sel[64 + n, n * DM:(n + 1) * DM] = 1.0
    return {
        "wz": wz.astype(np.float32),
        "wxa": wxa.astype(np.float32),
        "wxp": wxp.astype(np.float32),
        "wdt": wdt.astype(ml_dtypes.bfloat16),
        "wo": wo.astype(np.float32),
        "vecs": vecs,
        "ident": np.eye(DM, dtype=np.float32),
        "sel": sel.astype(ml_dtypes.bfloat16),
    }


def build_program(layers=LAYERS):
    import concourse.bass as bass
    import concourse.tile as tile
    from concourse import bacc, mybir
    from contextlib import ExitStack

    f32 = mybir.dt.float32
    bf16 = mybir.dt.bfloat16
    DT = {F32: f32, BF16: bf16}
    AF = mybir.ActivationFunctionType
    OP = mybir.AluOpType

    nc = bacc.Bacc("TRN2")

    xT = nc.dram_tensor("xT", [DM, L + 3], f32, kind="ExternalInput")
    wz_d = nc.dram_tensor("wz", [LAYERS, DM, DI], f32, kind="ExternalInput")
    wxa_d = nc.dram_tensor("wxa", [LAYERS, DM, DCONV * DI], f32, kind="ExternalInput")
    wxp_d = nc.dram_tensor("wxp", [LAYERS, DM, NDH * 96], f32, kind="ExternalInput")
    wdt_d = nc.dram_tensor("wdt", [LAYERS, RNK, DI], bf16, kind="ExternalInput")
    wo_d = nc.dram_tensor("wo", [LAYERS, DM, NDH * DM], f32, kind="ExternalInput")
    vecs_d = nc.dram_tensor("vecs", [LAYERS, DM, 6], f32, kind="ExternalInput")
    ident_d = nc.dram_tensor("ident", [DM, DM], f32, kind="ExternalInput")
    sel_d = nc.dram_tensor("sel", [80, NST * DM], bf16, kind="ExternalInput")
    out_d = nc.dram_tensor("out", [DM, L], f32, kind="ExternalOutput")

    with tile.TileContext(nc) as tc, ExitStack() as ctx:
        pers = ctx.enter_context(tc.tile_pool(name="pers", bufs=1))
        wts = ctx.enter_context(tc.tile_pool(name="wts", bufs=2))
        work = ctx.enter_context(tc.tile_pool(name="work", bufs=2))
        ps = ctx.enter_context(tc.tile_pool(name="ps", bufs=4, space="PSUM"))
        psacc = ctx.enter_context(tc.tile_pool(name="psacc", bufs=1, space="PSUM"))

        xt = pers.tile([DM, L + 3], f32, tag="xt", name="xt")
        nc.sync.dma_start(xt[:], xT[:])
        ident = pers.tile([DM, DM], f32, tag="ident", name="ident")
        nc.sync.dma_start(ident[:], ident_d[:])
        sel = pers.tile([80, NST * DM], bf16, tag="sel", name="sel")
        nc.sync.dma_start(sel[:], sel_d[:])

        xa = [pers.tile([DM, L], f32, tag=f"xa{dh}", name=f"xa{dh}") for dh in range(NDH)]
        dts = [pers.tile([DM, L], DT[DT_DT], tag=f"dt{dh}", name=f"dt{dh}") for dh in range(NDH)]
        dtu = [pers.tile([DM, L], DT[DTU_DT], tag=f"dtu{dh}", name=f"dtu{dh}") for dh in range(NDH)]
        sz = [pers.tile([DM, L], DT[SZ_DT], tag=f"sz{dh}", name=f"sz{dh}") for dh in range(NDH)]
        # pjs holds the x_proj outputs: dtraw @ rows 0:8, Bm @ 32:48, Cm @ 64:80
        pjs = pers.tile([96, L], bf16, tag="pjs", name="pjs")
        hlast = pers.tile([DM, NDH * NST], f32, tag="hlast", name="hlast")

        for layer in range(layers):
            wl = layer % LAYERS
            # ---- per-layer weights -> SBUF (double-buffered pool) ----
            w_z = wts.tile([DM, DI], f32, tag="w_z", name="w_z")
            nc.sync.dma_start(w_z[:], wz_d[wl])
            w_xa = wts.tile([DM, DCONV * DI], f32, tag="w_xa", name="w_xa")
            nc.sync.dma_start(w_xa[:], wxa_d[wl])
            w_xp = wts.tile([DM, NDH * 96], f32, tag="w_xp", name="w_xp")
            nc.sync.dma_start(w_xp[:], wxp_d[wl])
            w_dt = wts.tile([RNK, DI], bf16, tag="w_dt", name="w_dt")
            nc.sync.dma_start(w_dt[:], wdt_d[wl])
            w_o = wts.tile([DM, NDH * DM], f32, tag="w_o", name="w_o")
            nc.sync.dma_start(w_o[:], wo_d[wl])
            vec = wts.tile([DM, 6], f32, tag="vec", name="vec")
            nc.sync.dma_start(vec[:], vecs_d[wl])

            # ---- stage A1: in_proj+conv, silu gates (Sigmoid table), x_proj ----
            for cc in range(NCH):
                t0 = cc * CH
                for dh in range(NDH):
                    mslc = slice(dh * DM, (dh + 1) * DM)
                    # z path: silu(z) = z * sigmoid(z)
                    p_z = ps.tile([DM, CH], f32, tag="rep", name="rep")
                    nc.tensor.matmul(p_z[:], w_z[:, mslc], xt[:, t0 + 3:t0 + 3 + CH],
                                     start=True, stop=True)
                    sg = work.tile([DM, CH], f32, tag="sg", name="sg")
                    nc.scalar.activation(sg[:], p_z[:], AF.Sigmoid)
                    nc.vector.tensor_tensor(sz[dh][:, t0:t0 + CH], p_z[:], sg[:],
                                            OP.mult)
                    # xa path: conv folded as 4 shifted accumulating matmuls
                    p_xa = ps.tile([DM, CH], f32, tag="rep", name="rep")
                    for k in range(DCONV):
                        nc.tensor.matmul(
                            p_xa[:], w_xa[:, k * DI + dh * DM:k * DI + (dh + 1) * DM],
                            xt[:, t0 + k:t0 + k + CH],
                            start=(k == 0), stop=(k == DCONV - 1))
                    ux = work.tile([DM, CH], f32, tag="ux", name="ux")
                    nc.scalar.activation(ux[:], p_xa[:], AF.Identity,
                                         bias=vec[:, 0 + dh:1 + dh])
                    sgx = work.tile([DM, CH], f32, tag="sg", name="sg")
                    nc.scalar.activation(sgx[:], p_xa[:], AF.Sigmoid,
                                         bias=vec[:, 0 + dh:1 + dh])
                    nc.vector.tensor_tensor(xa[dh][:, t0:t0 + CH], ux[:], sgx[:],
                                            OP.mult)
                # x_proj: [96, CH] -> split to dtraw/Bt/Ct (32-aligned PSUM reads)
                p_pj = ps.tile([96, CH], f32, tag="rep", name="rep")
                for dh in range(NDH):
                    nc.tensor.matmul(p_pj[:], w_xp[:, dh * 96:(dh + 1) * 96],
                                     xa[dh][:, t0:t0 + CH],
                                     start=(dh == 0), stop=(dh == NDH - 1))
                nc.scalar.copy(pjs[:, t0:t0 + CH], p_pj[:])
            # ---- stage A2: dt = softplus(...) via Exp+Ln, batched per function
            # so the ACT table set is loaded once per pass (Exp and Ln live in
            # different PWP table sets here).
            for cc in range(NCH):
                t0 = cc * CH
                for dh in range(NDH):
                    mslc = slice(dh * DM, (dh + 1) * DM)
                    p_dt = ps.tile([DM, CH], f32, tag="rep", name="rep")
                    nc.tensor.matmul(p_dt[:], w_dt[:, mslc], pjs[0:RNK, t0:t0 + CH],
                                     start=True, stop=True)
                    # dts <- exp(dt_raw@W + b), overwritten by Ln below
                    nc.scalar.activation(dts[dh][:, t0:t0 + CH], p_dt[:], AF.Exp,
                                         bias=vec[:, 2 + dh:3 + dh])
            for cc in range(NCH):
                t0 = cc * CH
                for dh in range(NDH):
                    nc.scalar.activation(dts[dh][:, t0:t0 + CH],
                                         dts[dh][:, t0:t0 + CH], AF.Ln, bias=1.0)
                    nc.vector.tensor_tensor(dtu[dh][:, t0:t0 + CH],
                                            dts[dh][:, t0:t0 + CH],
                                            xa[dh][:, t0:t0 + CH], OP.mult)

            # ---- stage B + C: scan per quarter ----
            for q in range(NQ):
                q0 = q * QCH * CH
                acc = [psacc.tile([DM, QCH * CH], f32, tag=f"acc{dh}", name=f"acc{dh}") for dh in range(NDH)]
                hprev = [[None] * NST for _ in range(NDH)]
                for n in range(NST):
                    for c in range(QCH):
                        t0 = q0 + c * CH
                        # replicate B_n, C_n rows across 128 partitions
                        # (K=16 selector matmul; operand bases stay at 0)
                        brep = ps.tile([DM, CH], f32, tag="rep", name="rep")
                        nc.tensor.matmul(brep[:], sel[32:32 + NST, n * DM:(n + 1) * DM],
                                         pjs[32:32 + NST, t0:t0 + CH],
                                         start=True, stop=True)
                        crep = ps.tile([DM, CH], f32, tag="rep", name="rep")
                        nc.tensor.matmul(crep[:], sel[64:64 + NST, n * DM:(n + 1) * DM],
                                         pjs[64:64 + NST, t0:t0 + CH],
                                         start=True, stop=True)
                        for dh in range(NDH):
                            at = work.tile([DM, CH], DT[A_DT], tag="a", name="a")
                            nc.scalar.activation(at[:], dts[dh][:, t0:t0 + CH], AF.Exp,
                                                 scale=-float(n + 1))
                            bt = work.tile([DM, CH], DT[BT_DT], tag="b", name="b")
                            nc.vector.tensor_tensor(bt[:], dtu[dh][:, t0:t0 + CH],
                                                    brep[:], OP.mult)
                            ht = work.tile([DM, CH], DT[H_DT], tag=f"h{dh}",
                                           name=f"h{dh}")
                            if c == 0:
                                init = hlast[:, dh * NST + n:dh * NST + n + 1] \
                                    if (q > 0) else 0.0
                            else:
                                init = hprev[dh][n][:, CH - 1:CH]
                            nc.vector.tensor_tensor_scan(ht[:], at[:], bt[:], init,
                                                         OP.mult, OP.add)
                            hprev[dh][n] = ht
                            tmp = work.tile([DM, CH], DT[TMP_DT], tag="tmp", name="tmp")
                            nc.vector.tensor_tensor(tmp[:], ht[:], crep[:], OP.mult)
                            nc.tensor.matmul(acc[dh][:, c * CH:(c + 1) * CH],
                                             ident[:], tmp[:],
                                             start=(n == 0), stop=(n == NST - 1))
                    for dh in range(NDH):
                        if q < NQ - 1:
                            nc.vector.tensor_copy(
                                hlast[:, dh * NST + n:dh * NST + n + 1],
                                hprev[dh][n][:, CH - 1:CH])
                # stage C for this quarter
                for c in range(QCH):
                    t0 = q0 + c * CH
                    ygs = []
                    for dh in range(NDH):
                        y2 = work.tile([DM, CH], f32, tag="y2", name="y2")
                        nc.vector.scalar_tensor_tensor(
                            y2[:], xa[dh][:, t0:t0 + CH], vec[:, 4 + dh:5 + dh],
                            acc[dh][:, c * CH:(c + 1) * CH], OP.mult, OP.add)
                        yg = work.tile([DM, CH], f32, tag="yg", name="yg")
                        nc.vector.tensor_tensor(yg[:], y2[:], sz[dh][:, t0:t0 + CH],
                                                OP.mult)
                        ygs.append(yg)
                    p_x = ps.tile([DM, CH], f32, tag="rep", name="rep")
                    for dh in range(NDH):
                        nc.tensor.matmul(p_x[:], w_o[:, dh * DM:(dh + 1) * DM],
                                         ygs[dh][:], start=(dh == 0), stop=(dh == NDH - 1))
                    if layer < layers - 1:
                        nc.scalar.copy(xt[:, t0 + 3:t0 + 3 + CH], p_x[:])
                    else:
                        ot = work.tile([DM, CH], f32, tag="ot", name="ot")
                        nc.scalar.copy(ot[:], p_x[:])
                        nc.sync.dma_start(out_d[:, t0:t0 + CH], ot[:])
    nc.compile()
    return nc


def numpy_sim(inputs):
    """Tile-level numpy simulation of the exact device algorithm."""
    w = prep_weights(inputs)
    x = inputs["x"]  # [B, L, DM]
    out = np.empty((B, L, DM), np.float32)

    def q(v, dt):
        if dt == BF16:
            import ml_dtypes
            return v.astype(ml_dtypes.bfloat16).astype(np.float32)
        return v.astype(np.float32)

    def silu(v):
        return v / (1 + np.exp(-v))

    for bb in range(B):
        xt = np.zeros((DM, L + 3), np.float32)
        xt[:, 3:] = x[bb].T
        for layer in range(LAYERS):
            vec = w["vecs"][layer]
            xa, dts, dtu_, sz_ = [], [], [], []
            for dh in range(NDH):
                mslc = slice(dh * DM, (dh + 1) * DM)
                zp = w["wz"][layer][:, mslc].T @ xt[:, 3:]
                sz_.append(q(silu(zp), SZ_DT))
                pxa = np.zeros((DM, L), np.float32)
                for k in range(DCONV):
                    pxa += w["wxa"][layer][:, k * DI + dh * DM:k * DI + (dh + 1) * DM].T \
                        @ xt[:, k:k + L]
                xa.append(silu(pxa + vec[:, 0 + dh:1 + dh]))
            proj = np.zeros((96, L), np.float32)
            for dh in range(NDH):
                proj += w["wxp"][layer][:, dh * 96:(dh + 1) * 96].T @ xa[dh]
            dtraw = q(proj[0:RNK], BF16)
            Btl = q(proj[32:32 + NST], BF16)
            Ctl = q(proj[64:64 + NST], BF16)
            wdt_f = np.asarray(w["wdt"][layer], np.float32)
            for dh in range(NDH):
                mslc = slice(dh * DM, (dh + 1) * DM)
                pdt = wdt_f[:, mslc].T @ dtraw
                e = q(np.exp(pdt + vec[:, 2 + dh:3 + dh]), DT_DT)
                dts.append(q(np.log1p(e), DT_DT))
                dtu_.append(q(dts[dh] * xa[dh], DTU_DT))
            ys = []
            for dh in range(NDH):
                acc = np.zeros((DM, L), np.float32)
                for n in range(NST):
                    a = q(np.exp(-(n + 1) * dts[dh]), A_DT)
                    bt = q(dtu_[dh] * Btl[n:n + 1], BT_DT)
                    h = np.zeros((DM, L), np.float32)
                    s = np.zeros(DM, np.float32)
                    for t in range(L):
                        s = a[:, t] * s + bt[:, t]
                        h[:, t] = s
                    h = q(h, H_DT)
                    acc += q(h * Ctl[n:n + 1], TMP_DT)
                y2 = xa[dh] * vec[:, 4 + dh:5 + dh] + acc
                ys.append(y2 * sz_[dh])
            px = np.zeros((DM, L), np.float32)
            for dh in range(NDH):
                px += w["wo"][layer][:, dh * DM:(dh + 1) * DM].T @ ys[dh]
            xt[:, 3:] = px
        out[bb] = xt[:, 3:].T
    return out


_last_results = None


def kernel(**inputs):
    global _last_results
    from concourse.bass_utils import run_bass_kernel_spmd

    w = prep_weights(inputs)
    x = inputs["x"]
    nc = build_program()
    in_maps = []
    for bb in range(NCORES):
        xt = np.zeros((DM, L + 3), np.float32)
        xt[:, 3:] = x[bb].T
        m = {"xT": xt}
        m.update(w)
        in_maps.append(m)
    # the axon NTFF hook is absent in this container; never trace here
    os.environ["BASS_NEVER_TRACE"] = "1"
    br = run_bass_kernel_spmd(nc, in_maps, core_ids=list(range(NCORES)),
                              trace=False)
    _last_results = br
    out = np.empty((B, L, DM), np.float32)
    for bb in range(NCORES):
        out[bb] = br.results[bb]["out"].T
    return out



# revision 24
# speedup vs baseline: 15.5140x; 15.5140x over previous
"""Mamba-core (4-layer) Trainium2 Bass kernel, v3.

Sharding: data-parallel over batch B=8 across 8 NeuronCores (one sample per
core, zero collectives).  Per core, activations live in SBUF in
[feature, time] layout.

Key design points (vs the 2.48 ms/core v1 baseline):
  - fp32r (TF32-style) matmuls everywhere on the activation path: 1 cyc/col
    on the PE like bf16, but ~19-bit mantissa, so the recurrent error
    amplification that bf16 weights/activations cause is gone.  The scan
    path (B/C rows, dt, b, h, tmp) is bf16: those errors average out across
    the 16-state readout sum (measured 6e-5 end-to-end).
  - B_n/C_n rows are interleaved in the x_proj output (row 32+2n = B_n,
    33+2n = C_n), bounced through an HBM scratch tile, and replicated
    across partitions with ONE stride-0 broadcast DMA per (n, q) chunk.
    This keeps all DVE operands in SBUF (2x_1p bf16 mode) and frees the PE
    from replication matmuls.
  - The selective scan runs on the Pool/GPSIMD engine (tensor_tensor_scan,
    0.83 ns/elem); DVE does the bf16 multiplies (0.55 ns/elem); ACT does
    the a = exp(-(n+1)dt) decays; PE does projections + the identity-matmul
    readout accumulation into PSUM.
  - Layer body is a q-major pipeline (A1 -> A2 -> scan -> readout per
    1024-wide quarter) so stage-A work of quarter q+1 overlaps the scan of
    quarter q and no engine drains at stage boundaries.
  - conv bias lands in PSUM via a K=1 ones-matmul so silu needs only one
    Sigmoid + one DVE multiply; dt softplus is Exp+Ln from one ACT table
    set (softplus has no table on TRN2).
"""

import os
import numpy as np

DM = 128        # d_model
DI = 256        # d_inner
NDH = 2         # d_inner halves of 128
NST = 16        # d_state
RNK = 8         # dt_rank
L = 4096
LAYERS = 4
DCONV = 4
MMC = 512       # matmul/psum chunk
QW = 1024       # scan/vector chunk width
NQ = L // QW    # 4
QMC = QW // MMC  # 2
B = 8
NCORES = 8

# engine split knobs (n in 0..15)
SCAN_POOL_N = 0    # scans with n < SCAN_POOL_N run on Pool (HW: scan is DVE-only)
TMP_POOL_N = 16    # tmp-mults with n >= 16-TMP_POOL_N run on Pool
BT_POOL_N = 14     # bt-mults with n >= 16-BT_POOL_N run on Pool
ACHAIN = 2         # decays for the last ACHAIN states via a_n = a_{n-1} * a_0
COPIES_ON_POOL = False  # psum->sbuf copies (pjs/xt/ot) on Pool instead of ACT


def prep_weights(inputs):
    """Host-side weight preprocessing (numpy, tiny)."""
    import ml_dtypes
    bf = ml_dtypes.bfloat16
    in_w = inputs["in_proj_w"]    # [4, 512, 128]
    cw = inputs["conv_w"]         # [4, 256, 4]
    cb = inputs["conv_b"]         # [4, 256]
    xp_w = inputs["x_proj_w"]     # [4, 40, 256]
    dtp_w = inputs["dt_proj_w"]   # [4, 256, 8]
    dtp_b = inputs["dt_proj_b"]   # [4, 256]
    Dp = inputs["D"]              # [4, 256]
    out_w = inputs["out_proj_w"]  # [4, 128, 256]

    wz = np.ascontiguousarray(np.transpose(in_w[:, DI:, :], (0, 2, 1)))  # [4,128,256]
    # conv folded into in_proj: wxa[l, kd, k*DI+m] = cw[l, m, k] * in_w[l, m, kd]
    wxa = np.einsum("lmk,lmd->ldkm", cw, in_w[:, :DI, :])                # [4,128,4,256]
    wxa = np.ascontiguousarray(wxa.reshape(LAYERS, DM, DCONV * DI))
    # wxp[l, ksub, dh*64 + seg]: x_proj output rows: dtraw @ 0:8,
    # B_n @ 32+2n, C_n @ 33+2n (interleaved for the paired broadcast DMA)
    wxp_t = np.transpose(xp_w.reshape(LAYERS, 40, NDH, DM), (0, 3, 2, 1))  # [l,ksub,dh,40]
    wxp = np.zeros((LAYERS, DM, NDH, 64), np.float32)
    wxp[:, :, :, 0:RNK] = wxp_t[:, :, :, 0:RNK]
    wxp[:, :, :, 32:64:2] = wxp_t[:, :, :, RNK:RNK + NST]
    wxp[:, :, :, 33:64:2] = wxp_t[:, :, :, RNK + NST:RNK + 2 * NST]
    wxp = np.ascontiguousarray(wxp.reshape(LAYERS, DM, NDH * 64))
    wdt = np.ascontiguousarray(np.transpose(dtp_w, (0, 2, 1)))           # [4,8,256]
    # wo[l, ksub, dh*128+m] = out_w[l, m, dh*128+ksub]
    wo = np.transpose(out_w.reshape(LAYERS, DM, NDH, DM), (0, 3, 2, 1))
    wo = np.ascontiguousarray(wo.reshape(LAYERS, DM, NDH * DM))
    vecs = np.zeros((LAYERS, DM, 4), np.float32)
    for dh in range(NDH):
        s = slice(dh * DM, (dh + 1) * DM)
        vecs[:, :, 0 + dh] = dtp_b[:, s]
        vecs[:, :, 2 + dh] = Dp[:, s]
    # conv bias as a K=1 matmul lhsT row
    cbt = cb.reshape(LAYERS, 1, DI)
    return {
        "wz": wz.astype(np.float32),
        "wxa": wxa.astype(np.float32),
        "wxp": wxp.astype(np.float32),
        "wdt": wdt.astype(bf),
        "wo": wo.astype(np.float32),
        "vecs": vecs.astype(np.float32),
        "cbt": np.ascontiguousarray(cbt).astype(np.float32),
        "ident": np.eye(DM, dtype=np.float32).astype(bf),
        "ones": np.ones((1, MMC), np.float32),
    }


def build_program(layers=LAYERS, reps=1):
    import concourse.bass as bass
    import concourse.tile as tile
    from concourse import bacc, mybir
    from contextlib import ExitStack

    f32 = mybir.dt.float32
    bf16 = mybir.dt.bfloat16
    f32r = mybir.dt.float32r
    AF = mybir.ActivationFunctionType
    OP = mybir.AluOpType

    nc = bacc.Bacc("TRN2")

    xT = nc.dram_tensor("xT", [DM, L + 3], f32r, kind="ExternalInput")
    wz_d = nc.dram_tensor("wz", [LAYERS, DM, DI], f32r, kind="ExternalInput")
    wxa_d = nc.dram_tensor("wxa", [LAYERS, DM, DCONV * DI], f32r, kind="ExternalInput")
    wxp_d = nc.dram_tensor("wxp", [LAYERS, DM, NDH * 64], f32r, kind="ExternalInput")
    wdt_d = nc.dram_tensor("wdt", [LAYERS, RNK, DI], bf16, kind="ExternalInput")
    wo_d = nc.dram_tensor("wo", [LAYERS, DM, NDH * DM], f32r, kind="ExternalInput")
    vecs_d = nc.dram_tensor("vecs", [LAYERS, DM, 4], f32, kind="ExternalInput")
    cbt_d = nc.dram_tensor("cbt", [LAYERS, 1, DI], f32r, kind="ExternalInput")
    ident_d = nc.dram_tensor("ident", [DM, DM], bf16, kind="ExternalInput")
    ones_d = nc.dram_tensor("ones", [1, MMC], f32r, kind="ExternalInput")
    out_d = nc.dram_tensor("out", [DM, L], f32, kind="ExternalOutput")

    with tile.TileContext(nc) as tc, ExitStack() as ctx:
        pers = ctx.enter_context(tc.tile_pool(name="pers", bufs=1))
        wts = ctx.enter_context(tc.tile_pool(name="wts", bufs=2))
        work = ctx.enter_context(tc.tile_pool(name="work", bufs=3))
        bc = ctx.enter_context(tc.tile_pool(name="bc", bufs=3))
        ps = ctx.enter_context(tc.tile_pool(name="ps", bufs=2, space="PSUM"))
        psacc = ctx.enter_context(tc.tile_pool(name="psacc", bufs=1, space="PSUM"))
        dramp = ctx.enter_context(tc.tile_pool(name="dramp", bufs=2, space="DRAM"))

        ident = pers.tile([DM, DM], bf16, tag="ident", name="ident")
        nc.sync.dma_start(ident[:], ident_d[:])
        ones = pers.tile([1, MMC], f32r, tag="ones", name="ones")
        nc.sync.dma_start(ones[:], ones_d[:])

        xt = pers.tile([DM, L + 3], f32r, tag="xt", name="xt")
        xa = [pers.tile([DM, L], f32r, tag=f"xa{dh}", name=f"xa{dh}") for dh in range(NDH)]
        dts = [pers.tile([DM, L], bf16, tag=f"dt{dh}", name=f"dt{dh}") for dh in range(NDH)]
        dtu = [pers.tile([DM, L], bf16, tag=f"dtu{dh}", name=f"dtu{dh}") for dh in range(NDH)]
        sz = [pers.tile([DM, L], f32, tag=f"sz{dh}", name=f"sz{dh}") for dh in range(NDH)]
        # pjs: x_proj outputs: dtraw @ rows 0:8, B_n @ 32+2n, C_n @ 33+2n
        pjs = pers.tile([64, L], bf16, tag="pjs", name="pjs")
        hlast = pers.tile([DM, NDH * NST], f32, tag="hlast", name="hlast")

        def act_copy(dst, src_ap):
            if COPIES_ON_POOL:
                nc.gpsimd.tensor_copy(dst, src_ap)
            else:
                nc.scalar.copy(dst, src_ap)

        def load_weights(wl):
            W = {}
            W["w_z"] = wts.tile([DM, DI], f32r, tag="w_z", name="w_z")
            nc.sync.dma_start(W["w_z"][:], wz_d[wl])
            W["w_xa"] = wts.tile([DM, DCONV * DI], f32r, tag="w_xa", name="w_xa")
            nc.sync.dma_start(W["w_xa"][:], wxa_d[wl])
            W["w_xp"] = wts.tile([DM, NDH * 64], f32r, tag="w_xp", name="w_xp")
            nc.sync.dma_start(W["w_xp"][:], wxp_d[wl])
            W["w_dt"] = wts.tile([RNK, DI], bf16, tag="w_dt", name="w_dt")
            nc.sync.dma_start(W["w_dt"][:], wdt_d[wl])
            W["w_o"] = wts.tile([DM, NDH * DM], f32r, tag="w_o", name="w_o")
            nc.sync.dma_start(W["w_o"][:], wo_d[wl])
            W["vec"] = wts.tile([DM, 4], f32, tag="vec", name="vec")
            nc.sync.dma_start(W["vec"][:], vecs_d[wl])
            W["w_cb"] = wts.tile([1, DI], f32r, tag="w_cb", name="w_cb")
            nc.sync.dma_start(W["w_cb"][:], cbt_d[wl])
            W["scr"] = dramp.tile([2 * NST, L], bf16, tag="scr", name="scr")
            return W

        def stage_a1(W, q):
            q0 = q * QW
            # ---- stage A1(q): in_proj+conv, silu gates, x_proj ----
            for cc in range(QMC):
                t0 = q0 + cc * MMC
                for dh in range(NDH):
                    mslc = slice(dh * DM, (dh + 1) * DM)
                    # z path: silu(z) = z * sigmoid(z)
                    p_z = ps.tile([DM, MMC], f32, tag="pa", name="pz")
                    nc.tensor.matmul(p_z[:], W["w_z"][:, mslc],
                                     xt[:, t0 + 3:t0 + 3 + MMC],
                                     start=True, stop=True)
                    sg = work.tile([DM, MMC], f32, tag="sg", name="sg", bufs=2)
                    nc.scalar.activation(sg[:], p_z[:], AF.Sigmoid)
                    nc.vector.tensor_tensor(sz[dh][:, t0:t0 + MMC], p_z[:],
                                            sg[:], OP.mult)
                    # xa path: conv folded as bias-matmul + 4 shifted matmuls
                    p_xa = ps.tile([DM, MMC], f32, tag="pb", name="pxa")
                    nc.tensor.matmul(p_xa[:], W["w_cb"][:, mslc], ones[:],
                                     start=True, stop=False)
                    for k in range(DCONV):
                        nc.tensor.matmul(
                            p_xa[:],
                            W["w_xa"][:, k * DI + dh * DM:k * DI + (dh + 1) * DM],
                            xt[:, t0 + k:t0 + k + MMC],
                            start=False, stop=(k == DCONV - 1))
                    sgx = work.tile([DM, MMC], f32, tag="sgx", name="sgx", bufs=2)
                    nc.scalar.activation(sgx[:], p_xa[:], AF.Sigmoid)
                    nc.vector.tensor_tensor(xa[dh][:, t0:t0 + MMC], p_xa[:],
                                            sgx[:], OP.mult)
                # x_proj -> pjs (dtraw @ 0:8, B/C interleaved @ 32:64)
                p_pj = ps.tile([64, MMC], f32, tag="pa", name="ppj")
                for dh in range(NDH):
                    nc.tensor.matmul(p_pj[:], W["w_xp"][:, dh * 64:(dh + 1) * 64],
                                     xa[dh][:, t0:t0 + MMC],
                                     start=(dh == 0), stop=(dh == NDH - 1))
                act_copy(pjs[:, t0:t0 + MMC], p_pj[:])
            # B/C rows of this quarter -> HBM scratch (broadcast source)
            nc.sync.dma_start(W["scr"][:, q0:q0 + QW], pjs[32:64, q0:q0 + QW])

        def stage_a2(W, q):
            q0 = q * QW
            # ---- stage A2(q): dt = softplus via Exp+Ln; dtu ----
            for cc in range(QMC):
                t0 = q0 + cc * MMC
                for dh in range(NDH):
                    mslc = slice(dh * DM, (dh + 1) * DM)
                    p_dt = ps.tile([DM, MMC], f32, tag="pa", name="pdt")
                    nc.tensor.matmul(p_dt[:], W["w_dt"][:, mslc],
                                     pjs[0:RNK, t0:t0 + MMC],
                                     start=True, stop=True)
                    nc.scalar.activation(dts[dh][:, t0:t0 + MMC], p_dt[:],
                                         AF.Exp, bias=W["vec"][:, 0 + dh:1 + dh])
            for dh in range(NDH):
                nc.scalar.activation(dts[dh][:, q0:q0 + QW],
                                     dts[dh][:, q0:q0 + QW], AF.Ln, bias=1.0)
                nc.vector.tensor_tensor(dtu[dh][:, q0:q0 + QW],
                                        dts[dh][:, q0:q0 + QW],
                                        xa[dh][:, q0:q0 + QW], OP.mult)

        def stage_b(W, q):
            q0 = q * QW
            # ---- stage B(q): selective scan ----
            acc = [psacc.tile([DM, QW], f32, tag=f"acc{dh}", name=f"acc{dh}")
                   for dh in range(NDH)]
            a0s, aprev = [None] * NDH, [None] * NDH
            for n in range(NST):
                # replicate B_n|C_n across partitions: one broadcast DMA
                bcrep = bc.tile([DM, 2 * QW], bf16, tag="bcrep", name="bcrep")
                nc.sync.dma_start(
                    bcrep[:],
                    W["scr"][2 * n:2 * n + 2, q0:q0 + QW].partition_broadcast(DM))
                for dh in range(NDH):
                    if n >= NST - ACHAIN:
                        # a_n = a_{n-1} * a_0  (exp(-(n+1)dt) = r^{n+1})
                        at = work.tile([DM, QW], bf16, tag="a", name="a")
                        nc.vector.tensor_tensor(at[:], aprev[dh][:], a0s[dh][:],
                                                OP.mult)
                    elif n == 0:
                        at = work.tile([DM, QW], bf16, tag="a0", name="a0",
                                       bufs=2)
                        nc.scalar.activation(at[:], dts[dh][:, q0:q0 + QW],
                                             AF.Exp, scale=-1.0)
                        a0s[dh] = at
                    else:
                        at = work.tile([DM, QW], bf16, tag="a", name="a")
                        nc.scalar.activation(at[:], dts[dh][:, q0:q0 + QW],
                                             AF.Exp, scale=-float(n + 1))
                    aprev[dh] = at
                    bt = work.tile([DM, QW], bf16, tag="b", name="b")
                    bt_eng = nc.gpsimd if n >= NST - BT_POOL_N else nc.vector
                    bt_eng.tensor_tensor(bt[:], dtu[dh][:, q0:q0 + QW],
                                         bcrep[:, 0:QW], OP.mult)
                    ht = work.tile([DM, QW], bf16, tag="h", name="h")
                    init = 0.0 if q == 0 else hlast[:, dh * NST + n:dh * NST + n + 1]
                    scan_eng = nc.gpsimd if n < SCAN_POOL_N else nc.vector
                    scan_eng.tensor_tensor_scan(ht[:], at[:], bt[:], init,
                                                OP.mult, OP.add)
                    if q < NQ - 1:
                        nc.gpsimd.tensor_copy(
                            hlast[:, dh * NST + n:dh * NST + n + 1],
                            ht[:, QW - 1:QW])
                    tmp = work.tile([DM, QW], bf16, tag="tmp", name="tmp")
                    tmp_eng = nc.gpsimd if n >= NST - TMP_POOL_N else nc.vector
                    tmp_eng.tensor_tensor(tmp[:], ht[:], bcrep[:, QW:2 * QW],
                                          OP.mult)
                    for c4 in range(QMC):
                        nc.tensor.matmul(
                            acc[dh][:, c4 * MMC:(c4 + 1) * MMC],
                            ident[:], tmp[:, c4 * MMC:(c4 + 1) * MMC],
                            start=(n == 0), stop=(n == NST - 1))
            return acc

        def stage_c(W, q, acc, last):
            q0 = q * QW
            # ---- stage C(q): gate + out_proj ----
            for c4 in range(QMC):
                s0 = q0 + c4 * MMC
                ygs = []
                for dh in range(NDH):
                    y2 = work.tile([DM, MMC], f32, tag="y2", name="y2")
                    nc.vector.scalar_tensor_tensor(
                        y2[:], xa[dh][:, s0:s0 + MMC], W["vec"][:, 2 + dh:3 + dh],
                        acc[dh][:, c4 * MMC:(c4 + 1) * MMC], OP.mult, OP.add)
                    yg = work.tile([DM, MMC], f32r, tag="yg", name="yg")
                    nc.vector.tensor_tensor(yg[:], y2[:], sz[dh][:, s0:s0 + MMC],
                                            OP.mult)
                    ygs.append(yg)
                p_x = ps.tile([DM, MMC], f32, tag="pb", name="px")
                for dh in range(NDH):
                    nc.tensor.matmul(p_x[:], W["w_o"][:, dh * DM:(dh + 1) * DM],
                                     ygs[dh][:], start=(dh == 0),
                                     stop=(dh == NDH - 1))
                if not last:
                    act_copy(xt[:, s0 + 3:s0 + 3 + MMC], p_x[:])
                else:
                    ot = work.tile([DM, MMC], f32, tag="ot", name="ot")
                    act_copy(ot[:], p_x[:])
                    nc.sync.dma_start(out_d[:, s0:s0 + MMC], ot[:])

        def body():
            # Cross-layer software pipeline: layer l+1's stage A1(q) is
            # emitted right after layer l's stage C(q) (which produced the
            # xt columns A1 needs), so no engine drains at layer borders.
            # A1 emission also precedes the NEXT C's xt overwrite of its
            # 3 boundary columns (in-place xt, write-after-read).
            nc.sync.dma_start(xt[:], xT[:])
            Wcur = load_weights(0)
            for q in range(NQ):
                stage_a1(Wcur, q)
            for layer in range(layers):
                last = layer == layers - 1
                Wnext = None if last else load_weights((layer + 1) % LAYERS)
                stage_a2(Wcur, 0)
                acc_prev = stage_b(Wcur, 0)
                for q in range(1, NQ):
                    stage_a2(Wcur, q)
                    stage_c(Wcur, q - 1, acc_prev, last)
                    if not last:
                        stage_a1(Wnext, q - 1)
                    acc_prev = stage_b(Wcur, q)
                stage_c(Wcur, NQ - 1, acc_prev, last)
                if not last:
                    stage_a1(Wnext, NQ - 1)
                Wcur = Wnext

        if reps == 1:
            body()
        else:
            with tc.For_i(0, reps) as _i:
                body()
    nc.compile()
    return nc


def make_in_map(inputs, w, bb):
    x = inputs["x"]
    xt = np.zeros((DM, L + 3), np.float32)
    xt[:, 3:] = x[bb].T
    m = {"xT": xt}
    m.update(w)
    return m


_scan_jit = None


def _np_scan(a, bt):
    """h[:, t] = a[:, t] * h[:, t-1] + bt[:, t], fp32 (jax.lax.scan, jitted)."""
    global _scan_jit
    import jax
    import jax.numpy as jnp
    if _scan_jit is None:
        def f(a_, b_):
            def step(s, ab):
                s = ab[0] * s + ab[1]
                return s, s
            _, h = jax.lax.scan(step, jnp.zeros(a_.shape[0], jnp.float32),
                                (a_.T, b_.T))
            return h.T
        _scan_jit = jax.jit(f, backend="cpu")
    return np.asarray(_scan_jit(a, bt))


def numpy_sim(inputs, layers=LAYERS):
    """Tile-level numpy simulation of the exact device algorithm."""
    import ml_dtypes
    bfq = lambda v: v.astype(ml_dtypes.bfloat16).astype(np.float32)
    w = prep_weights(inputs)
    wf = {k: np.asarray(v, np.float32) for k, v in w.items()}
    x = inputs["x"]
    out = np.empty((B, L, DM), np.float32)

    for bb in range(B):
        xt = np.zeros((DM, L + 3), np.float32)
        xt[:, 3:] = x[bb].T
        for layer in range(layers):
            wl = layer % LAYERS
            vec = wf["vecs"][wl]
            xa, dts, dtu_, sz_ = [], [], [], []
            for dh in range(NDH):
                mslc = slice(dh * DM, (dh + 1) * DM)
                zp = wf["wz"][wl][:, mslc].T @ xt[:, 3:]
                sz_.append(zp * (1 / (1 + np.exp(-zp))))
                pxa = np.broadcast_to(wf["cbt"][wl][0, mslc][:, None], (DM, L)).copy()
                for k in range(DCONV):
                    pxa += wf["wxa"][wl][:, k * DI + dh * DM:k * DI + (dh + 1) * DM].T \
                        @ xt[:, k:k + L]
                xa.append(pxa * (1 / (1 + np.exp(-pxa))))
            proj = np.zeros((64, L), np.float32)
            for dh in range(NDH):
                proj += wf["wxp"][wl][:, dh * 64:(dh + 1) * 64].T @ xa[dh]
            pjs = bfq(proj)
            Btl, Ctl = pjs[32:64:2], pjs[33:64:2]
            for dh in range(NDH):
                mslc = slice(dh * DM, (dh + 1) * DM)
                pdt = wf["wdt"][wl][:, mslc].T @ pjs[0:RNK]
                e = bfq(np.exp(pdt + vec[:, 0 + dh:1 + dh]))
                dts.append(bfq(np.log1p(e)))
                dtu_.append(bfq(dts[dh] * xa[dh]))
            ys = []
            for dh in range(NDH):
                acc = np.zeros((DM, L), np.float32)
                a0 = aprev = None
                for n in range(NST):
                    if n >= NST - ACHAIN:
                        a = bfq(aprev * a0)
                    else:
                        a = bfq(np.exp(-(n + 1) * dts[dh]))
                        if n == 0:
                            a0 = a
                    aprev = a
                    bt = bfq(dtu_[dh] * Btl[n:n + 1])
                    h = bfq(_np_scan(a, bt))
                    acc += bfq(h * Ctl[n:n + 1])
                y2 = xa[dh] * vec[:, 2 + dh:3 + dh] + acc
                ys.append(y2 * sz_[dh])
            px = np.zeros((DM, L), np.float32)
            for dh in range(NDH):
                px += wf["wo"][wl][:, dh * DM:(dh + 1) * DM].T @ ys[dh]
            xt[:, 3:] = px
        out[bb] = xt[:, 3:].T
    return out


_last_results = None


def kernel(**inputs):
    global _last_results
    from concourse.bass_utils import run_bass_kernel_spmd

    w = prep_weights(inputs)
    nc = build_program()
    in_maps = [make_in_map(inputs, w, bb) for bb in range(NCORES)]
    # the axon NTFF hook is absent in this container; never trace here
    os.environ["BASS_NEVER_TRACE"] = "1"
    br = run_bass_kernel_spmd(nc, in_maps, core_ids=list(range(NCORES)),
                              trace=False)
    _last_results = br
    out = np.empty((B, L, DM), np.float32)
    for bb in range(NCORES):
        out[bb] = br.results[bb]["out"].T
    return out


# revision 26
# speedup vs baseline: 25.7541x; 1.6601x over previous
"""Mamba-core (4-layer) Trainium2 Bass kernel, v3.

Sharding: data-parallel over batch B=8 across 8 NeuronCores (one sample per
core, zero collectives).  Per core, activations live in SBUF in
[feature, time] layout.

Key design points (vs the 2.48 ms/core v1 baseline):
  - fp32r (TF32-style) matmuls everywhere on the activation path: 1 cyc/col
    on the PE like bf16, but ~19-bit mantissa, so the recurrent error
    amplification that bf16 weights/activations cause is gone.  The scan
    path (B/C rows, dt, b, h, tmp) is bf16: those errors average out across
    the 16-state readout sum (measured 6e-5 end-to-end).
  - B_n/C_n rows are interleaved in the x_proj output (row 32+2n = B_n,
    33+2n = C_n), bounced through an HBM scratch tile, and replicated
    across partitions with ONE stride-0 broadcast DMA per (n, q) chunk.
    This keeps all DVE operands in SBUF (2x_1p bf16 mode) and frees the PE
    from replication matmuls.
  - The selective scan runs on the Pool/GPSIMD engine (tensor_tensor_scan,
    0.83 ns/elem); DVE does the bf16 multiplies (0.55 ns/elem); ACT does
    the a = exp(-(n+1)dt) decays; PE does projections + the identity-matmul
    readout accumulation into PSUM.
  - Layer body is a q-major pipeline (A1 -> A2 -> scan -> readout per
    1024-wide quarter) so stage-A work of quarter q+1 overlaps the scan of
    quarter q and no engine drains at stage boundaries.
  - conv bias lands in PSUM via a K=1 ones-matmul so silu needs only one
    Sigmoid + one DVE multiply; dt softplus is Exp+Ln from one ACT table
    set (softplus has no table on TRN2).
"""

import os
import numpy as np

DM = 128        # d_model
DI = 256        # d_inner
NDH = 2         # d_inner halves of 128
NST = 16        # d_state
RNK = 8         # dt_rank
L = 4096
LAYERS = 4
DCONV = 4
MMC = 512       # matmul/psum chunk
QW = 1024       # scan/vector chunk width
NQ = L // QW    # 4
QMC = QW // MMC  # 2
B = 8
NCORES = 8

# engine split knobs (n in 0..15)
# HW-measured: tensor_tensor_scan is DVE-only (ISA rejects Pool), and ANY
# GPSIMD tensor work degrades DVE throughput via shared SBUF ports, so all
# element-wise work stays on DVE (Pool does only the tiny hlast copies).
SCAN_POOL_N = 0    # scans with n < SCAN_POOL_N run on Pool (HW: DVE-only)
TMP_POOL_N = 0     # tmp-mults with n >= 16-TMP_POOL_N run on Pool
BT_POOL_N = 0      # bt-mults with n >= 16-BT_POOL_N run on Pool
ACHAIN = 2         # decays for the last ACHAIN states via a_n = a_{n-1} * a_0
SINGLE_BC = False  # timing probe: load bcrep once per quarter (wrong values)
COPIES_ON_POOL = False  # psum->sbuf copies (pjs/xt/ot) on Pool instead of ACT


def prep_weights(inputs):
    """Host-side weight preprocessing (numpy, tiny)."""
    import ml_dtypes
    bf = ml_dtypes.bfloat16
    in_w = inputs["in_proj_w"]    # [4, 512, 128]
    cw = inputs["conv_w"]         # [4, 256, 4]
    cb = inputs["conv_b"]         # [4, 256]
    xp_w = inputs["x_proj_w"]     # [4, 40, 256]
    dtp_w = inputs["dt_proj_w"]   # [4, 256, 8]
    dtp_b = inputs["dt_proj_b"]   # [4, 256]
    Dp = inputs["D"]              # [4, 256]
    out_w = inputs["out_proj_w"]  # [4, 128, 256]

    wz = np.ascontiguousarray(np.transpose(in_w[:, DI:, :], (0, 2, 1)))  # [4,128,256]
    # conv folded into in_proj: wxa[l, kd, k*DI+m] = cw[l, m, k] * in_w[l, m, kd]
    wxa = np.einsum("lmk,lmd->ldkm", cw, in_w[:, :DI, :])                # [4,128,4,256]
    wxa = np.ascontiguousarray(wxa.reshape(LAYERS, DM, DCONV * DI))
    # wxp[l, ksub, dh*64 + seg]: x_proj output rows: dtraw @ 0:8,
    # B_n @ 32+2n, C_n @ 33+2n (interleaved for the paired broadcast DMA)
    wxp_t = np.transpose(xp_w.reshape(LAYERS, 40, NDH, DM), (0, 3, 2, 1))  # [l,ksub,dh,40]
    wxp = np.zeros((LAYERS, DM, NDH, 64), np.float32)
    wxp[:, :, :, 0:RNK] = wxp_t[:, :, :, 0:RNK]
    wxp[:, :, :, 32:64:2] = wxp_t[:, :, :, RNK:RNK + NST]
    wxp[:, :, :, 33:64:2] = wxp_t[:, :, :, RNK + NST:RNK + 2 * NST]
    wxp = np.ascontiguousarray(wxp.reshape(LAYERS, DM, NDH * 64))
    wdt = np.ascontiguousarray(np.transpose(dtp_w, (0, 2, 1)))           # [4,8,256]
    # wo[l, ksub, dh*128+m] = out_w[l, m, dh*128+ksub]
    wo = np.transpose(out_w.reshape(LAYERS, DM, NDH, DM), (0, 3, 2, 1))
    wo = np.ascontiguousarray(wo.reshape(LAYERS, DM, NDH * DM))
    vecs = np.zeros((LAYERS, DM, 4), np.float32)
    for dh in range(NDH):
        s = slice(dh * DM, (dh + 1) * DM)
        vecs[:, :, 0 + dh] = dtp_b[:, s]
        vecs[:, :, 2 + dh] = Dp[:, s]
    # conv bias as a K=1 matmul lhsT row
    cbt = cb.reshape(LAYERS, 1, DI)
    return {
        "wz": wz.astype(np.float32),
        "wxa": wxa.astype(np.float32),
        "wxp": wxp.astype(np.float32),
        "wdt": wdt.astype(bf),
        "wo": wo.astype(np.float32),
        "vecs": vecs.astype(np.float32),
        "cbt": np.ascontiguousarray(cbt).astype(np.float32),
        "ident": np.eye(DM, dtype=np.float32).astype(bf),
        "ones": np.ones((1, MMC), np.float32),
    }


def build_program(layers=LAYERS, reps=1):
    import concourse.bass as bass
    import concourse.tile as tile
    from concourse import bacc, mybir
    from contextlib import ExitStack

    f32 = mybir.dt.float32
    bf16 = mybir.dt.bfloat16
    f32r = mybir.dt.float32r
    AF = mybir.ActivationFunctionType
    OP = mybir.AluOpType

    nc = bacc.Bacc("TRN2")

    xT = nc.dram_tensor("xT", [DM, L + 3], f32r, kind="ExternalInput")
    wz_d = nc.dram_tensor("wz", [LAYERS, DM, DI], f32r, kind="ExternalInput")
    wxa_d = nc.dram_tensor("wxa", [LAYERS, DM, DCONV * DI], f32r, kind="ExternalInput")
    wxp_d = nc.dram_tensor("wxp", [LAYERS, DM, NDH * 64], f32r, kind="ExternalInput")
    wdt_d = nc.dram_tensor("wdt", [LAYERS, RNK, DI], bf16, kind="ExternalInput")
    wo_d = nc.dram_tensor("wo", [LAYERS, DM, NDH * DM], f32r, kind="ExternalInput")
    vecs_d = nc.dram_tensor("vecs", [LAYERS, DM, 4], f32, kind="ExternalInput")
    cbt_d = nc.dram_tensor("cbt", [LAYERS, 1, DI], f32r, kind="ExternalInput")
    ident_d = nc.dram_tensor("ident", [DM, DM], bf16, kind="ExternalInput")
    ones_d = nc.dram_tensor("ones", [1, MMC], f32r, kind="ExternalInput")
    out_d = nc.dram_tensor("out", [DM, L], f32, kind="ExternalOutput")

    with tile.TileContext(nc) as tc, ExitStack() as ctx:
        pers = ctx.enter_context(tc.tile_pool(name="pers", bufs=1))
        wts = ctx.enter_context(tc.tile_pool(name="wts", bufs=2))
        work = ctx.enter_context(tc.tile_pool(name="work", bufs=3))
        bc = ctx.enter_context(tc.tile_pool(name="bc", bufs=3))
        ps = ctx.enter_context(tc.tile_pool(name="ps", bufs=2, space="PSUM"))
        psacc = ctx.enter_context(tc.tile_pool(name="psacc", bufs=1, space="PSUM"))
        dramp = ctx.enter_context(tc.tile_pool(name="dramp", bufs=2, space="DRAM"))

        ident = pers.tile([DM, DM], bf16, tag="ident", name="ident")
        nc.sync.dma_start(ident[:], ident_d[:])
        ones = pers.tile([1, MMC], f32r, tag="ones", name="ones")
        nc.sync.dma_start(ones[:], ones_d[:])

        xt = pers.tile([DM, L + 3], f32r, tag="xt", name="xt")
        xa = [pers.tile([DM, L], f32r, tag=f"xa{dh}", name=f"xa{dh}") for dh in range(NDH)]
        dts = [pers.tile([DM, L], bf16, tag=f"dt{dh}", name=f"dt{dh}") for dh in range(NDH)]
        dtu = [pers.tile([DM, L], bf16, tag=f"dtu{dh}", name=f"dtu{dh}") for dh in range(NDH)]
        sz = [pers.tile([DM, L], f32, tag=f"sz{dh}", name=f"sz{dh}") for dh in range(NDH)]
        # pjs: x_proj outputs: dtraw @ rows 0:8, B_n @ 32+2n, C_n @ 33+2n
        pjs = pers.tile([64, L], bf16, tag="pjs", name="pjs")
        hlast = pers.tile([DM, NDH * NST], f32, tag="hlast", name="hlast")

        def act_copy(dst, src_ap):
            if COPIES_ON_POOL:
                nc.gpsimd.tensor_copy(dst, src_ap)
            else:
                nc.scalar.copy(dst, src_ap)

        def load_weights(wl):
            W = {}
            W["w_z"] = wts.tile([DM, DI], f32r, tag="w_z", name="w_z")
            nc.sync.dma_start(W["w_z"][:], wz_d[wl])
            W["w_xa"] = wts.tile([DM, DCONV * DI], f32r, tag="w_xa", name="w_xa")
            nc.sync.dma_start(W["w_xa"][:], wxa_d[wl])
            W["w_xp"] = wts.tile([DM, NDH * 64], f32r, tag="w_xp", name="w_xp")
            nc.sync.dma_start(W["w_xp"][:], wxp_d[wl])
            W["w_dt"] = wts.tile([RNK, DI], bf16, tag="w_dt", name="w_dt")
            nc.sync.dma_start(W["w_dt"][:], wdt_d[wl])
            W["w_o"] = wts.tile([DM, NDH * DM], f32r, tag="w_o", name="w_o")
            nc.sync.dma_start(W["w_o"][:], wo_d[wl])
            W["vec"] = wts.tile([DM, 4], f32, tag="vec", name="vec")
            nc.sync.dma_start(W["vec"][:], vecs_d[wl])
            W["w_cb"] = wts.tile([1, DI], f32r, tag="w_cb", name="w_cb")
            nc.sync.dma_start(W["w_cb"][:], cbt_d[wl])
            W["scr"] = dramp.tile([2 * NST, L], bf16, tag="scr", name="scr")
            return W

        def stage_a1(W, q):
            q0 = q * QW
            # ---- stage A1(q): in_proj+conv, silu gates, x_proj ----
            for cc in range(QMC):
                t0 = q0 + cc * MMC
                for dh in range(NDH):
                    mslc = slice(dh * DM, (dh + 1) * DM)
                    # z path: silu(z) = z * sigmoid(z)
                    p_z = ps.tile([DM, MMC], f32, tag="pa", name="pz")
                    nc.tensor.matmul(p_z[:], W["w_z"][:, mslc],
                                     xt[:, t0 + 3:t0 + 3 + MMC],
                                     start=True, stop=True)
                    sg = work.tile([DM, MMC], f32, tag="sg", name="sg", bufs=2)
                    nc.scalar.activation(sg[:], p_z[:], AF.Sigmoid)
                    nc.vector.tensor_tensor(sz[dh][:, t0:t0 + MMC], p_z[:],
                                            sg[:], OP.mult)
                    # xa path: conv folded as bias-matmul + 4 shifted matmuls
                    p_xa = ps.tile([DM, MMC], f32, tag="pb", name="pxa")
                    nc.tensor.matmul(p_xa[:], W["w_cb"][:, mslc], ones[:],
                                     start=True, stop=False)
                    for k in range(DCONV):
                        nc.tensor.matmul(
                            p_xa[:],
                            W["w_xa"][:, k * DI + dh * DM:k * DI + (dh + 1) * DM],
                            xt[:, t0 + k:t0 + k + MMC],
                            start=False, stop=(k == DCONV - 1))
                    sgx = work.tile([DM, MMC], f32, tag="sgx", name="sgx", bufs=2)
                    nc.scalar.activation(sgx[:], p_xa[:], AF.Sigmoid)
                    nc.vector.tensor_tensor(xa[dh][:, t0:t0 + MMC], p_xa[:],
                                            sgx[:], OP.mult)
                # x_proj -> pjs (dtraw @ 0:8, B/C interleaved @ 32:64)
                p_pj = ps.tile([64, MMC], f32, tag="pa", name="ppj")
                for dh in range(NDH):
                    nc.tensor.matmul(p_pj[:], W["w_xp"][:, dh * 64:(dh + 1) * 64],
                                     xa[dh][:, t0:t0 + MMC],
                                     start=(dh == 0), stop=(dh == NDH - 1))
                act_copy(pjs[:, t0:t0 + MMC], p_pj[:])
            # B/C rows of this quarter -> HBM scratch (broadcast source)
            nc.sync.dma_start(W["scr"][:, q0:q0 + QW], pjs[32:64, q0:q0 + QW])

        def stage_a2(W, q):
            q0 = q * QW
            # ---- stage A2(q): dt = softplus via Exp+Ln; dtu ----
            for cc in range(QMC):
                t0 = q0 + cc * MMC
                for dh in range(NDH):
                    mslc = slice(dh * DM, (dh + 1) * DM)
                    p_dt = ps.tile([DM, MMC], f32, tag="pa", name="pdt")
                    nc.tensor.matmul(p_dt[:], W["w_dt"][:, mslc],
                                     pjs[0:RNK, t0:t0 + MMC],
                                     start=True, stop=True)
                    nc.scalar.activation(dts[dh][:, t0:t0 + MMC], p_dt[:],
                                         AF.Exp, bias=W["vec"][:, 0 + dh:1 + dh])
            for dh in range(NDH):
                nc.scalar.activation(dts[dh][:, q0:q0 + QW],
                                     dts[dh][:, q0:q0 + QW], AF.Ln, bias=1.0)
                nc.vector.tensor_tensor(dtu[dh][:, q0:q0 + QW],
                                        dts[dh][:, q0:q0 + QW],
                                        xa[dh][:, q0:q0 + QW], OP.mult)

        def stage_b(W, q):
            q0 = q * QW
            # ---- stage B(q): selective scan ----
            acc = [psacc.tile([DM, QW], f32, tag=f"acc{dh}", name=f"acc{dh}")
                   for dh in range(NDH)]
            a0s, aprev = [None] * NDH, [None] * NDH
            bc_shared = None
            for n in range(NST):
                # replicate B_n|C_n across partitions: one broadcast DMA
                if SINGLE_BC and n > 0:
                    bcrep = bc_shared
                else:
                    bcrep = bc.tile([DM, 2 * QW], bf16, tag="bcrep", name="bcrep")
                    nc.sync.dma_start(
                        bcrep[:],
                        W["scr"][2 * n:2 * n + 2, q0:q0 + QW].partition_broadcast(DM))
                    bc_shared = bcrep
                for dh in range(NDH):
                    if n >= NST - ACHAIN:
                        # a_n = a_{n-1} * a_0  (exp(-(n+1)dt) = r^{n+1})
                        at = work.tile([DM, QW], bf16, tag="a", name="a")
                        nc.vector.tensor_tensor(at[:], aprev[dh][:], a0s[dh][:],
                                                OP.mult)
                    elif n == 0:
                        at = work.tile([DM, QW], bf16, tag="a0", name="a0",
                                       bufs=2)
                        nc.scalar.activation(at[:], dts[dh][:, q0:q0 + QW],
                                             AF.Exp, scale=-1.0)
                        a0s[dh] = at
                    else:
                        at = work.tile([DM, QW], bf16, tag="a", name="a")
                        nc.scalar.activation(at[:], dts[dh][:, q0:q0 + QW],
                                             AF.Exp, scale=-float(n + 1))
                    aprev[dh] = at
                    bt = work.tile([DM, QW], bf16, tag="b", name="b")
                    bt_eng = nc.gpsimd if n >= NST - BT_POOL_N else nc.vector
                    bt_eng.tensor_tensor(bt[:], dtu[dh][:, q0:q0 + QW],
                                         bcrep[:, 0:QW], OP.mult)
                    ht = work.tile([DM, QW], bf16, tag="h", name="h")
                    init = 0.0 if q == 0 else hlast[:, dh * NST + n:dh * NST + n + 1]
                    scan_eng = nc.gpsimd if n < SCAN_POOL_N else nc.vector
                    scan_eng.tensor_tensor_scan(ht[:], at[:], bt[:], init,
                                                OP.mult, OP.add)
                    if q < NQ - 1:
                        nc.gpsimd.tensor_copy(
                            hlast[:, dh * NST + n:dh * NST + n + 1],
                            ht[:, QW - 1:QW])
                    tmp = work.tile([DM, QW], bf16, tag="tmp", name="tmp")
                    tmp_eng = nc.gpsimd if n >= NST - TMP_POOL_N else nc.vector
                    tmp_eng.tensor_tensor(tmp[:], ht[:], bcrep[:, QW:2 * QW],
                                          OP.mult)
                    for c4 in range(QMC):
                        nc.tensor.matmul(
                            acc[dh][:, c4 * MMC:(c4 + 1) * MMC],
                            ident[:], tmp[:, c4 * MMC:(c4 + 1) * MMC],
                            start=(n == 0), stop=(n == NST - 1))
            return acc

        def stage_c(W, q, acc, last):
            q0 = q * QW
            # ---- stage C(q): gate + out_proj ----
            for c4 in range(QMC):
                s0 = q0 + c4 * MMC
                ygs = []
                for dh in range(NDH):
                    y2 = work.tile([DM, MMC], f32, tag="y2", name="y2")
                    nc.vector.scalar_tensor_tensor(
                        y2[:], xa[dh][:, s0:s0 + MMC], W["vec"][:, 2 + dh:3 + dh],
                        acc[dh][:, c4 * MMC:(c4 + 1) * MMC], OP.mult, OP.add)
                    yg = work.tile([DM, MMC], f32r, tag="yg", name="yg")
                    nc.vector.tensor_tensor(yg[:], y2[:], sz[dh][:, s0:s0 + MMC],
                                            OP.mult)
                    ygs.append(yg)
                p_x = ps.tile([DM, MMC], f32, tag="pb", name="px")
                for dh in range(NDH):
                    nc.tensor.matmul(p_x[:], W["w_o"][:, dh * DM:(dh + 1) * DM],
                                     ygs[dh][:], start=(dh == 0),
                                     stop=(dh == NDH - 1))
                if not last:
                    act_copy(xt[:, s0 + 3:s0 + 3 + MMC], p_x[:])
                else:
                    ot = work.tile([DM, MMC], f32, tag="ot", name="ot")
                    act_copy(ot[:], p_x[:])
                    nc.sync.dma_start(out_d[:, s0:s0 + MMC], ot[:])

        def body():
            # Cross-layer software pipeline: layer l+1's stage A1(q) is
            # emitted right after layer l's stage C(q) (which produced the
            # xt columns A1 needs), so no engine drains at layer borders.
            # A1 emission also precedes the NEXT C's xt overwrite of its
            # 3 boundary columns (in-place xt, write-after-read).
            nc.sync.dma_start(xt[:], xT[:])
            Wcur = load_weights(0)
            for q in range(NQ):
                stage_a1(Wcur, q)
            for layer in range(layers):
                last = layer == layers - 1
                Wnext = None if last else load_weights((layer + 1) % LAYERS)
                stage_a2(Wcur, 0)
                acc_prev = stage_b(Wcur, 0)
                for q in range(1, NQ):
                    stage_a2(Wcur, q)
                    stage_c(Wcur, q - 1, acc_prev, last)
                    if not last:
                        stage_a1(Wnext, q - 1)
                    acc_prev = stage_b(Wcur, q)
                stage_c(Wcur, NQ - 1, acc_prev, last)
                if not last:
                    stage_a1(Wnext, NQ - 1)
                Wcur = Wnext

        if reps == 1:
            body()
        else:
            with tc.For_i(0, reps) as _i:
                body()
    nc.compile()
    return nc


def make_in_map(inputs, w, bb):
    x = inputs["x"]
    xt = np.zeros((DM, L + 3), np.float32)
    xt[:, 3:] = x[bb].T
    m = {"xT": xt}
    m.update(w)
    return m


_scan_jit = None


def _np_scan(a, bt):
    """h[:, t] = a[:, t] * h[:, t-1] + bt[:, t], fp32 (jax.lax.scan, jitted)."""
    global _scan_jit
    import jax
    import jax.numpy as jnp
    if _scan_jit is None:
        def f(a_, b_):
            def step(s, ab):
                s = ab[0] * s + ab[1]
                return s, s
            _, h = jax.lax.scan(step, jnp.zeros(a_.shape[0], jnp.float32),
                                (a_.T, b_.T))
            return h.T
        _scan_jit = jax.jit(f, backend="cpu")
    return np.asarray(_scan_jit(a, bt))


def numpy_sim(inputs, layers=LAYERS):
    """Tile-level numpy simulation of the exact device algorithm."""
    import ml_dtypes
    bfq = lambda v: v.astype(ml_dtypes.bfloat16).astype(np.float32)
    w = prep_weights(inputs)
    wf = {k: np.asarray(v, np.float32) for k, v in w.items()}
    x = inputs["x"]
    out = np.empty((B, L, DM), np.float32)

    for bb in range(B):
        xt = np.zeros((DM, L + 3), np.float32)
        xt[:, 3:] = x[bb].T
        for layer in range(layers):
            wl = layer % LAYERS
            vec = wf["vecs"][wl]
            xa, dts, dtu_, sz_ = [], [], [], []
            for dh in range(NDH):
                mslc = slice(dh * DM, (dh + 1) * DM)
                zp = wf["wz"][wl][:, mslc].T @ xt[:, 3:]
                sz_.append(zp * (1 / (1 + np.exp(-zp))))
                pxa = np.broadcast_to(wf["cbt"][wl][0, mslc][:, None], (DM, L)).copy()
                for k in range(DCONV):
                    pxa += wf["wxa"][wl][:, k * DI + dh * DM:k * DI + (dh + 1) * DM].T \
                        @ xt[:, k:k + L]
                xa.append(pxa * (1 / (1 + np.exp(-pxa))))
            proj = np.zeros((64, L), np.float32)
            for dh in range(NDH):
                proj += wf["wxp"][wl][:, dh * 64:(dh + 1) * 64].T @ xa[dh]
            pjs = bfq(proj)
            Btl, Ctl = pjs[32:64:2], pjs[33:64:2]
            for dh in range(NDH):
                mslc = slice(dh * DM, (dh + 1) * DM)
                pdt = wf["wdt"][wl][:, mslc].T @ pjs[0:RNK]
                e = bfq(np.exp(pdt + vec[:, 0 + dh:1 + dh]))
                dts.append(bfq(np.log1p(e)))
                dtu_.append(bfq(dts[dh] * xa[dh]))
            ys = []
            for dh in range(NDH):
                acc = np.zeros((DM, L), np.float32)
                a0 = aprev = None
                for n in range(NST):
                    if n >= NST - ACHAIN:
                        a = bfq(aprev * a0)
                    else:
                        a = bfq(np.exp(-(n + 1) * dts[dh]))
                        if n == 0:
                            a0 = a
                    aprev = a
                    bt = bfq(dtu_[dh] * Btl[n:n + 1])
                    h = bfq(_np_scan(a, bt))
                    acc += bfq(h * Ctl[n:n + 1])
                y2 = xa[dh] * vec[:, 2 + dh:3 + dh] + acc
                ys.append(y2 * sz_[dh])
            px = np.zeros((DM, L), np.float32)
            for dh in range(NDH):
                px += wf["wo"][wl][:, dh * DM:(dh + 1) * DM].T @ ys[dh]
            xt[:, 3:] = px
        out[bb] = xt[:, 3:].T
    return out


_last_results = None


def kernel(**inputs):
    global _last_results
    from concourse.bass_utils import run_bass_kernel_spmd

    w = prep_weights(inputs)
    nc = build_program()
    in_maps = [make_in_map(inputs, w, bb) for bb in range(NCORES)]
    # the axon NTFF hook is absent in this container; never trace here
    os.environ["BASS_NEVER_TRACE"] = "1"
    br = run_bass_kernel_spmd(nc, in_maps, core_ids=list(range(NCORES)),
                              trace=False)
    _last_results = br
    out = np.empty((B, L, DM), np.float32)
    for bb in range(NCORES):
        out[bb] = br.results[bb]["out"].T
    return out
